# revision 1
# baseline (speedup 1.0000x reference)
"""Encoder kernel library: builds the full Bass/Tile program for
nn_Encoder (watermark encoder) on 8 TRN2 cores, data-parallel over batch.

Layout conventions per core (1 image):
  activations y_k in DRAM as [64, 256, 256] (pre-BN, conv bias added)
  enc in DRAM as [h, c, w] = [256, 64, 256]
  lpn in DRAM as [3, 256, 256]  (holds -floor(clip(low,0,255)) ; conv0 weights negated/255)
"""
import numpy as np
import concourse.bass as bass
import concourse.tile as tile
from concourse import bacc, mybir
from contextlib import ExitStack

f32 = mybir.dt.float32
f32r = mybir.dt.float32r
FT = mybir.ActivationFunctionType
ALU = mybir.AluOpType

H = W = 256
HW = H * W
CH = 64
MSG = 30
WP = W + 2          # padded row length 258
R = 32              # conv band rows
NBAND = H // R      # 8

# ---------------------------------------------------------------- host consts
def host_constants():
    j = np.arange(256)
    ang = 2.0 * np.pi * np.outer(j, j) / 256.0
    C = np.cos(ang).astype(np.float32)
    S = (-np.sin(ang)).astype(np.float32)      # F = C + iS
    Ci = (C / 256.0).astype(np.float32)
    Sq = (S / 256.0).astype(np.float32)
    consts = {
        "cC": C, "cS": S, "cNS": -S, "cCi": Ci, "cSq": Sq, "cNSq": -Sq,
        "cCS": np.hstack([C, S]).astype(np.float32),
        "cIdent": np.eye(128, dtype=np.float32),
    }
    # low-pass mask, ifftshifted, in [kw, kh] layout (symmetric anyway)
    yy = np.arange(H, dtype=np.float32) - H // 2
    xx = np.arange(W, dtype=np.float32) - W // 2
    m = ((yy[:, None] ** 2 + xx[None, :] ** 2) <= float(50 * 50)).astype(np.float32)
    consts["cMask"] = np.fft.ifftshift(m).astype(np.float32)
    # watermark position constants (all in channel 0; block kh,kw in 124..132)
    cy = cx = 128
    rr, cc = [], []
    idx = 0
    for i in range(-4, 5):
        for j2 in range(-4, 5):
            if idx >= MSG:
                break
            if (i * i + j2 * j2) ** 0.5 <= 4:
                rr.append(cy + i); cc.append(cx + j2); idx += 1
    rr = np.array(rr); cc = np.array(cc)   # rr = kh (axis -2), cc = kw (axis -1)
    pk = np.zeros((MSG, 9), np.float32)    # partition (kw) indicator
    fk = np.zeros((MSG, 9), np.float32)    # free (kh) indicator
    kap = np.zeros((9, 9), np.float32)
    for k in range(MSG):
        pk[k, cc[k] - 124] = 1.0
        fk[k, rr[k] - 124] = 1.0
        kap[cc[k] - 124, rr[k] - 124] = 1.0
    consts["cPk"] = pk
    consts["cFk"] = fk
    consts["cOnemk"] = (1.0 - kap)
    return consts


# ---------------------------------------------------------------- builders
class Enc:
    def __init__(self, n_cores=8, debug_outs=()):
        self.n_cores = n_cores
        self.ntot = float(n_cores * HW)
        self.debug_outs = debug_outs
        nc = bacc.Bacc("TRN2", target_bir_lowering=False, debug=False,
                       num_devices=n_cores)
        self.nc = nc
        d = {}
        d["image"] = nc.dram_tensor("image", (3, H, W), f32, kind="ExternalInput").ap()
        d["message"] = nc.dram_tensor("message", (MSG, 1), f32, kind="ExternalInput").ap()
        d["w0"] = nc.dram_tensor("w0", (64, 3, 3, 3), f32, kind="ExternalInput").ap()
        d["b0"] = nc.dram_tensor("b0", (64, 1), f32, kind="ExternalInput").ap()
        d["g0"] = nc.dram_tensor("g0", (64, 1), f32, kind="ExternalInput").ap()
        d["be0"] = nc.dram_tensor("be0", (64, 1), f32, kind="ExternalInput").ap()
        d["ws"] = nc.dram_tensor("ws", (3, 64, 64, 3, 3), f32, kind="ExternalInput").ap()
        d["bs"] = nc.dram_tensor("bs", (3, 64, 1), f32, kind="ExternalInput").ap()
        d["gs"] = nc.dram_tensor("gs", (3, 64, 1), f32, kind="ExternalInput").ap()
        d["bes"] = nc.dram_tensor("bes", (3, 64, 1), f32, kind="ExternalInput").ap()
        d["acw"] = nc.dram_tensor("acw", (64, 97, 3, 3), f32, kind="ExternalInput").ap()
        d["acb"] = nc.dram_tensor("acb", (64, 1), f32, kind="ExternalInput").ap()
        d["acg"] = nc.dram_tensor("acg", (64, 1), f32, kind="ExternalInput").ap()
        d["acbe"] = nc.dram_tensor("acbe", (64, 1), f32, kind="ExternalInput").ap()
        d["fw"] = nc.dram_tensor("fw", (3, 64), f32, kind="ExternalInput").ap()
        d["fb"] = nc.dram_tensor("fb", (3, 1), f32, kind="ExternalInput").ap()
        for k, shp in [("cC", (256, 256)), ("cS", (256, 256)), ("cNS", (256, 256)),
                       ("cCi", (256, 256)), ("cSq", (256, 256)), ("cNSq", (256, 256)),
                       ("cCS", (256, 512)),
                       ("cIdent", (128, 128)), ("cMask", (256, 256)),
                       ("cPk", (MSG, 9)), ("cFk", (MSG, 9)), ("cOnemk", (9, 9))]:
            d[k] = nc.dram_tensor(k, shp, f32, kind="ExternalInput").ap()
        d["out"] = nc.dram_tensor("out", (3, H, W), f32, kind="ExternalOutput").ap()
        self.d = d
        self.dbg = {}

    def maybe_debug(self, name, shape):
        """Declare an extra output for stage validation."""
        if name in self.debug_outs:
            self.dbg[name] = self.nc.dram_tensor(
                "dbg_" + name, shape, f32, kind="ExternalOutput").ap()
            return self.dbg[name]
        return None

    # ------------------------------------------------------------------
    def build(self):
        nc, d = self.nc, self.d
        with tile.TileContext(nc) as tc, ExitStack() as X:
            cp = X.enter_context(tc.tile_pool(name="consts", bufs=1))
            dp = X.enter_context(tc.tile_pool(name="dram", bufs=1, space="DRAM"))
            pwX = ExitStack()
            pw = pwX.enter_context(tc.tile_pool(name="pw", bufs=2, space="PSUM"))
            wev = pwX.enter_context(tc.tile_pool(name="wev", bufs=2))

            # ---------------- DRAM scratch
            y = [dp.tile([64, H, W], f32, name=f"yact{k}") for k in range(5)]
            lpn_d = dp.tile([3, H, W], f32, name="lpn_d")
            enc_d = dp.tile([H, 64, W], f32, name="enc_d")
            cl_in = [dp.tile([64, 2], f32, name=f"clin{k}") for k in range(5)]
            cl_out = [dp.tile([64, 2], f32, name=f"clout{k}", addr_space="Shared")
                      for k in range(5)]

            # ---------------- constants into SBUF
            def cload(name, src, shape, dtype):
                t = cp.tile(shape, dtype, name=name)
                nc.sync.dma_start(t[:], src if dtype == f32 else src.bitcast(dtype))
                return t
            # DFT matrices, f32r halves for main chain, f32 for lowpass chain
            DF = {}
            for nm, wdt in [("cC", 256), ("cS", 256), ("cNS", 256), ("cCi", 256),
                            ("cSq", 256), ("cNSq", 256), ("cCS", 512)]:
                DF[nm + "_hi"] = cload(nm + "_hi", d[nm][0:128, :], [128, wdt], f32r)
                DF[nm + "_lo"] = cload(nm + "_lo", d[nm][128:256, :], [128, wdt], f32r)
                DF[nm + "_hi32"] = cload(nm + "_hi32", d[nm][0:128, :], [128, wdt], f32)
                DF[nm + "_lo32"] = cload(nm + "_lo32", d[nm][128:256, :], [128, wdt], f32)
            ident = cload("ident", d["cIdent"][:], [128, 128], f32)
            maskt = [cload(f"maskt{i}", d["cMask"][i * 128:(i + 1) * 128, :],
                           [128, 256], f32) for i in range(2)]
            pk_t = cload("pk_t", d["cPk"][:], [MSG, 9], f32)
            fk_t = cload("fk_t", d["cFk"][:], [MSG, 9], f32)
            onemk_t = cload("onemk_t", d["cOnemk"][:], [9, 9], f32)
            msg_t = cload("msg_t", d["message"][:], [MSG, 1], f32)
            ones30 = cp.tile([MSG, 1, 256], f32, name="ones30")
            nc.vector.memset(ones30[:], 1.0)
            zero128 = cp.tile([128, 1], f32, name="zero128")
            nc.vector.memset(zero128[:], 0.0)
            eps64 = cp.tile([64, 1], f32, name="eps64")
            nc.vector.memset(eps64[:], 1e-5)

            # per-layer bn param tiles
            def vload(name, src):
                t = cp.tile([64, 1], f32, name=name)
                nc.sync.dma_start(t[:], src)
                return t
            g_t = [vload("g_t0", d["g0"][:])] + \
                  [vload(f"g_t{k+1}", d["gs"][k]) for k in range(3)] + \
                  [vload("g_t4", d["acg"][:])]
            be_t = [vload("be_t0", d["be0"][:])] + \
                   [vload(f"be_t{k+1}", d["bes"][k]) for k in range(3)] + \
                   [vload("be_t4", d["acbe"][:])]
            # conv bias replicated to 128 partitions (for double-chunk evict)
            b128 = []
            for k, src in enumerate([d["b0"], d["bs"][0], d["bs"][1], d["bs"][2],
                                     d["acb"]]):
                t = cp.tile([128, 1], f32, name=f"b128_{k}")
                nc.sync.dma_start(t[0:64, :], src)
                nc.sync.dma_start(t[64:128, :], src)
                b128.append(t)
            fb_t = cp.tile([3, 1], f32, name="fb_t")
            nc.sync.dma_start(fb_t[:], d["fb"][:])

            # ---------------- weight transposes (lhsT prep)
            # conv1..3: pair lhsT [128,64] x3(dw), single lhsT [64,64] x3
            lhsT_pair, lhsT_sing = [], []
            for k in range(3):
                wsrc = cp.tile([64, 64, 9], f32, name=f"wsrc{k}")
                nc.sync.dma_start(wsrc[:], d["ws"][k].rearrange("o i a b -> o i (a b)"))
                pairs, sings = [], []
                for dw in range(3):
                    pA = cp.tile([128, 128], f32r, name=f"lhsTpA{k}{dw}")
                    pB = cp.tile([128, 128], f32r, name=f"lhsTpB{k}{dw}")
                    cx = cp.tile([64, 128], f32r, name=f"lhsTcx{k}{dw}")
                    nc.vector.memset(pA[:].bitcast(f32), 0.0)
                    nc.vector.memset(pB[:].bitcast(f32), 0.0)
                    tp3 = []
                    for dh in range(3):
                        p = pw.tile([64, 64], f32, name="pwt")
                        nc.tensor.transpose(p[:], wsrc[:, :, dh * 3 + dw], ident[0:64, 0:64])
                        tp3.append(p)
                    # pairA: K=(dh-1,dh0), M cols 0:64  (upper chunk rows t,t+1)
                    nc.vector.tensor_copy(pA[0:64, 0:64], tp3[0][:])
                    nc.vector.tensor_copy(pA[64:128, 0:64], tp3[1][:])
                    # pairB: K=(dh0,dh+1), M cols 64:128 (lower chunk rows t+2,t+3)
                    nc.vector.tensor_copy(pB[0:64, 64:128], tp3[1][:])
                    nc.vector.tensor_copy(pB[64:128, 64:128], tp3[2][:])
                    # cross: K=64, M = [W(+1) | W(-1)]
                    nc.vector.tensor_copy(cx[:, 0:64], tp3[2][:])
                    nc.vector.tensor_copy(cx[:, 64:128], tp3[0][:])
                    pairs.append((pA, pB)); sings.append(cx)
                lhsT_pair.append(pairs); lhsT_sing.append(sings)
            # conv0: lhsT0[dw] [9,64] = -w0[:,ci,dh,dw]^T/255 stacked over dh
            w0src = cp.tile([64, 3, 9], f32, name="w0src")
            nc.sync.dma_start(w0src[:], d["w0"][:].rearrange("o i a b -> o i (a b)"))
            lhsT0 = []
            for dw in range(3):
                l0 = cp.tile([18, 128], f32r, name=f"lhsT0{dw}")
                nc.vector.memset(l0[:].bitcast(f32), 0.0)
                for dh in range(3):
                    p = pw.tile([64, 64], f32, name="pwt")
                    nc.tensor.transpose(p[0:3, :], w0src[:, :, dh * 3 + dw],
                                        ident[0:64, 0:64])
                    tmp0 = wev.tile([3, 64], f32r, name="w0tmp")
                    nc.vector.tensor_scalar_mul(tmp0[:], p[0:3, :], -1.0 / 255.0)
                    nc.sync.dma_start(l0[dh * 3:dh * 3 + 3, 0:64], tmp0[:])
                    nc.sync.dma_start(l0[9 + dh * 3:9 + dh * 3 + 3, 64:128], tmp0[:])
                lhsT0.append(l0)
            # ac conv: lhsT_ac[dh*3+dw] [97,64]
            acsrc = cp.tile([64, 97, 9], f32, name="acsrc")
            nc.sync.dma_start(acsrc[:], d["acw"][:].rearrange("o i a b -> o i (a b)"))
            lhsT_ac = []
            ac_tp = []
            for tap in range(9):
                la = [cp.tile([97, 128], f32r, name=f"lhsTac{tap}{v}") for v in range(2)]
                for v in range(2):
                    nc.vector.memset(la[v][:].bitcast(f32), 0.0)
                p = pw.tile([97, 64], f32, name="pwa", bufs=4)
                nc.tensor.transpose(p[:], acsrc[:, :, tap], ident[0:64, 0:64])
                for v in range(2):
                    nc.vector.tensor_copy(la[v][:, v * 64:v * 64 + 64], p[:])
                lhsT_ac.append(la)
                ac_tp.append(p)
            lhsT_accx = []
            for dw in range(3):
                cxa = cp.tile([97, 128], f32r, name=f"lhsTaccx{dw}")
                nc.vector.tensor_copy(cxa[:, 0:64], ac_tp[6 + dw][:])
                nc.vector.tensor_copy(cxa[:, 64:128], ac_tp[0 + dw][:])
                lhsT_accx.append(cxa)
            # final 1x1: lhsT_fin [64,3]
            fwsrc = cp.tile([3, 64], f32, name="fwsrc")
            nc.sync.dma_start(fwsrc[:], d["fw"][:])
            lhsT_fin = cp.tile([128, 6], f32r, name="lhsT_fin")
            nc.vector.memset(lhsT_fin[:].bitcast(f32), 0.0)
            p = pw.tile([64, 64], f32, name="pwt")
            nc.tensor.transpose(p[:, 0:3], fwsrc[:], ident[0:3, 0:3])
            nc.vector.tensor_copy(lhsT_fin[0:64, 0:3], p[:, 0:3])
            nc.vector.tensor_copy(lhsT_fin[64:128, 3:6], p[:, 0:3])

            # ---------------- watermark value prep
            msgc = cp.tile([MSG, 9], f32, name="msgc")
            nc.vector.tensor_scalar(msgc[:], fk_t[:], msg_t[:, 0:1], None, op0=ALU.mult)
            pwm = pw.tile([9, 9], f32, name="pwm")
            nc.tensor.matmul(pwm[:], pk_t[:], msgc[:], start=True, stop=True)
            wmv = cp.tile([9, 9], f32, name="wmv")
            nc.vector.tensor_copy(wmv[:], pwm[:])
            wm_al = [cp.tile([128, 9], f32, name=f"wm_al{i}") for i in range(2)]
            onemk_al = [cp.tile([128, 9], f32, name=f"onemk_al{i}") for i in range(2)]
            for i in range(2):
                nc.vector.memset(wm_al[i][:], 0.0)
                nc.vector.memset(onemk_al[i][:], 1.0)
            nc.sync.dma_start(wm_al[0][124:128, :], wmv[0:4, :])
            nc.sync.dma_start(wm_al[1][0:5, :], wmv[4:9, :])
            nc.sync.dma_start(onemk_al[0][124:128, :], onemk_t[0:4, :])
            nc.sync.dma_start(onemk_al[1][0:5, :], onemk_t[4:9, :])

            self._consts = dict(DF=DF, ident=ident, maskt=maskt, ones30=ones30,
                                g_t=g_t, be_t=be_t, b128=b128, fb_t=fb_t,
                                lhsT_pair=lhsT_pair, lhsT_sing=lhsT_sing,
                                lhsT0=lhsT0, lhsT_ac=lhsT_ac, lhsT_accx=lhsT_accx, lhsT_fin=lhsT_fin,
                                wm_al=wm_al, onemk_al=onemk_al, msg_sc=msg_t,
                                zero128=zero128, eps64=eps64, cp=cp,
                                y=y, lpn_d=lpn_d, enc_d=enc_d,
                                cl_in=cl_in, cl_out=cl_out)
            pwX.close()
            self._build_body(X, tc)
        self.nc.compile()

    # ------------------------------------------------------------------
    def _stats_finalize(self, tc, pool, layer, ssum_cols, sqsum_cols, ncols):
        """Reduce per-chunk stat columns, AllReduce, return (scale, shift) [64,1]."""
        nc = self.nc
        C = self._consts
        red = pool.tile([128, 2], f32, name=f"red{layer}")
        nc.vector.tensor_reduce(red[:, 0:1], ssum_cols[:, 0:ncols], axis=mybir.AxisListType.X, op=ALU.add)
        nc.vector.tensor_reduce(red[:, 1:2], sqsum_cols[:, 0:ncols], axis=mybir.AxisListType.X, op=ALU.add)
        upper = pool.tile([64, 2], f32, name=f"upper{layer}")
        nc.sync.dma_start(upper[:], red[64:128, :])
        stats = pool.tile([64, 2], f32, name=f"stats{layer}")
        nc.vector.tensor_add(stats[:], red[0:64, :], upper[:])
        nc.sync.dma_start(C["cl_in"][layer][:], stats[:])
        ap = self.maybe_debug(f"st{layer}", (64, 2))
        if ap is not None:
            nc.sync.dma_start(ap[:], stats[:])
        nc.gpsimd.collective_compute(
            "AllReduce", ALU.add,
            replica_groups=[list(range(self.n_cores))],
            ins=[C["cl_in"][layer].opt()], outs=[C["cl_out"][layer].opt()])
        sr = pool.tile([64, 2], f32, name=f"sr{layer}")
        nc.sync.dma_start(sr[:], C["cl_out"][layer][:])
        ap = self.maybe_debug(f"sr{layer}", (64, 2))
        if ap is not None:
            nc.sync.dma_start(ap[:], sr[:])
        mean = pool.tile([64, 1], f32, name=f"mean{layer}")
        nc.vector.tensor_scalar_mul(mean[:], sr[:, 0:1], 1.0 / self.ntot)
        ms = pool.tile([64, 1], f32, name=f"ms{layer}")
        nc.vector.tensor_scalar_mul(ms[:], sr[:, 1:2], 1.0 / self.ntot)
        msq = pool.tile([64, 1], f32, name=f"msq{layer}")
        nc.vector.tensor_scalar(msq[:], mean[:], mean[:, 0:1], None, op0=ALU.mult)
        var = pool.tile([64, 1], f32, name=f"var{layer}")
        nc.vector.tensor_scalar(var[:], ms[:], msq[:, 0:1], None, op0=ALU.subtract)
        std = pool.tile([64, 1], f32, name=f"std{layer}")
        nc.scalar.activation(std[:], var[:], FT.Sqrt, bias=C["eps64"][:, 0:1], scale=1.0)
        istd = pool.tile([64, 1], f32, name=f"istd{layer}")
        nc.vector.reciprocal(istd[:], std[:])
        scale = pool.tile([64, 1], f32, name=f"scale{layer}")
        nc.vector.tensor_tensor(scale[:], C["g_t"][layer][:], istd[:], op=ALU.mult)
        prod = pool.tile([64, 1], f32, name=f"prod{layer}")
        nc.vector.tensor_tensor(prod[:], mean[:], scale[:], op=ALU.mult)
        shift = pool.tile([64, 1], f32, name=f"shift{layer}")
        nc.vector.scalar_tensor_tensor(shift[:], prod[:], -1.0, C["be_t"][layer][:],
                                       op0=ALU.mult, op1=ALU.add)
        ap = self.maybe_debug(f"sc{layer}", (64, 1))
        if ap is not None:
            nc.sync.dma_start(ap[:], scale[:])
        ap = self.maybe_debug(f"sh{layer}", (64, 1))
        if ap is not None:
            nc.sync.dma_start(ap[:], shift[:])
        return scale, shift

    # ------------------------------------------------------------------
    def _conv64(self, X, tc, layer, src, dst, scale, shift):
        """conv layers 1..3 (64->64) and layer 4 (ac, 97->64) share this via flags."""
        nc = self.nc
        C = self._consts
        is_ac = (layer == 4)
        KP = 97 if is_ac else 128
        with ExitStack() as S:
            bp = S.enter_context(tc.tile_pool(name=f"band{layer}", bufs=3))
            pp = S.enter_context(tc.tile_pool(name=f"psum{layer}", bufs=3, space="PSUM"))
            ep = S.enter_context(tc.tile_pool(name=f"evict{layer}", bufs=3))
            sp = S.enter_context(tc.tile_pool(name=f"stat{layer}", bufs=1))
            ssum_cols = sp.tile([128, 64], f32, name=f"ssc{layer}")
            sqsum_cols = sp.tile([128, 64], f32, name=f"sqc{layer}")
            cidx = 0
            for bi in range(NBAND):
                r0 = bi * R
                band = bp.tile([KP, (R + 2) * WP], f32r, name=f"bandt{layer}")
                b3 = band[:].rearrange("p (r c) -> p r c", c=WP)
                # rows of image present in band: band row i = image row r0-1+i
                i0 = 1 if bi == 0 else 0
                i1 = R + 1 if bi == NBAND - 1 else R + 2
                rl, rh = r0 - 1 + i0, r0 - 1 + i1
                if is_ac:
                    nc.vector.memset(b3[0:30, :, :].bitcast(f32), 0.0)
                    nc.vector.memset(b3[0:97, :, 0:1].bitcast(f32), 0.0)
                    nc.vector.memset(b3[0:97, :, 257:258].bitcast(f32), 0.0)
                    if bi == 0:
                        nc.vector.memset(b3[0:97, 0:1, :].bitcast(f32), 0.0)
                    if bi == NBAND - 1:
                        nc.vector.memset(b3[0:97, R + 1:R + 2, :].bitcast(f32), 0.0)
                    # msg channels: interior = message value
                    nc.scalar.activation(
                        b3[0:30, i0:i1, 1:257],
                        C["ones30"][:].broadcast_to([MSG, i1 - i0, 256]),
                        FT.Copy, bias=0.0, scale=C["msg_sc"][:, 0:1])
                    # enc channels from enc_d [h,c,w]
                    nc.sync.dma_start(
                        b3[30:94, i0:i1, 1:257],
                        C["enc_d"][rl:rh, :, :].transpose([1, 0, 2]).bitcast(f32r))
                    # image channels
                    nc.sync.dma_start(b3[94:97, i0:i1, 1:257],
                                      self.d["image"][:, rl:rh, :].bitcast(f32r))
                else:
                    nc.vector.memset(b3[0:64, :, 0:1].bitcast(f32), 0.0)
                    nc.vector.memset(b3[0:64, :, 257:258].bitcast(f32), 0.0)
                    if bi == 0:
                        nc.vector.memset(b3[0:64, 0:1, :].bitcast(f32), 0.0)
                    if bi == NBAND - 1:
                        nc.vector.memset(b3[0:64, R + 1:R + 2, :].bitcast(f32), 0.0)
                    nc.sync.dma_start(b3[0:64, i0:i1, 1:257],
                                      src[:, rl:rh, :].bitcast(f32r))
                    nc.scalar.activation(b3[0:64, i0:i1, 1:257],
                                         b3[0:64, i0:i1, 1:257].bitcast(f32),
                                         FT.Relu, bias=shift[:, 0:1], scale=scale[:, 0:1])
                    # dup: partitions 64:128 = band shifted one row down
                    nc.sync.dma_start(b3[64:128, 0:R + 1, :],
                                      b3[0:64, 1:R + 2, :])
                for t in range(0, R, 4):
                    pt = pp.tile([128, 512], f32, name=f"pchunk{layer}")
                    if is_ac:
                        nmm_tot = 15
                        mm = 0
                        for dw in range(3):
                            for dh in range(2):       # upper taps dh=-1,0
                                nc.tensor.matmul(
                                    pt[:], C["lhsT_ac"][dh * 3 + dw][0][:],
                                    b3[0:97, t + dh:t + dh + 2, dw:dw + 256],
                                    start=(mm == 0), stop=(mm == nmm_tot - 1))
                                mm += 1
                            for dh in range(1, 3):    # lower taps dh=0,+1
                                nc.tensor.matmul(
                                    pt[:], C["lhsT_ac"][dh * 3 + dw][1][:],
                                    b3[0:97, t + 2 + dh:t + 4 + dh, dw:dw + 256],
                                    start=(mm == 0), stop=(mm == nmm_tot - 1))
                                mm += 1
                            nc.tensor.matmul(         # cross: upper +1, lower -1
                                pt[:], C["lhsT_accx"][dw][:],
                                b3[0:97, t + 2:t + 4, dw:dw + 256],
                                start=(mm == 0), stop=(mm == nmm_tot - 1))
                            mm += 1
                    else:
                        nmm_tot = 9
                        mm = 0
                        for dw in range(3):
                            pA, pB = C["lhsT_pair"][layer - 1][dw]
                            nc.tensor.matmul(
                                pt[:], pA[:], b3[0:128, t:t + 2, dw:dw + 256],
                                start=(mm == 0), stop=(mm == nmm_tot - 1))
                            mm += 1
                            nc.tensor.matmul(
                                pt[:], pB[:], b3[0:128, t + 3:t + 5, dw:dw + 256],
                                start=(mm == 0), stop=(mm == nmm_tot - 1))
                            mm += 1
                            nc.tensor.matmul(
                                pt[:], C["lhsT_sing"][layer - 1][dw][:],
                                b3[0:64, t + 2:t + 4, dw:dw + 256],
                                start=(mm == 0), stop=(mm == nmm_tot - 1))
                            mm += 1
                    ysb = ep.tile([128, 512], f32, name=f"ysb{layer}")
                    nc.vector.tensor_scalar(ysb[:], pt[:], C["b128"][layer][:, 0:1],
                                            0.0, op0=ALU.add, op1=ALU.add,
                                            accum_out=ssum_cols[:, cidx:cidx + 1])
                    scr = ep.tile([128, 512], f32, name=f"sqscr{layer}")
                    nc.scalar.activation(scr[:], ysb[:], FT.Square,
                                         bias=C["zero128"][:, 0:1],
                                         accum_out=sqsum_cols[:, cidx:cidx + 1])
                    cidx += 1
                    q = r0 + t
                    nc.sync.dma_start(dst[:, q:q + 2, :],
                                      ysb[0:64, :].rearrange("p (r c) -> p r c", c=256))
                    nc.sync.dma_start(dst[:, q + 2:q + 4, :],
                                      ysb[64:128, :].rearrange("p (r c) -> p r c", c=256))
            sc, sh = self._stats_finalize(tc, sp, layer, ssum_cols, sqsum_cols, cidx)
            return sc, sh

    # ------------------------------------------------------------------
    def _fft_chain(self, tc, pools, *, src_loader, dt, suffix32,
                   wm=False, mask=False, evict_fn=None):
        """Transpose-free fused fft2 -> edit -> ifft2 for ONE channel.

        All passes are normal matmuls (HAM-visible). Odd passes use the data
        as the stationary operand, producing transposed output for free.
        Layouts (per channel):
          x   [h, w]                     (xt tiles, 2 h-blocks)
          Zt  [w, (Zre|Zim)]  = x^T @ [C|S]          (P1, data-stationary)
          f   [kw, (fre|fim)] = Fw^T @ Zt            (P2, const-stationary)
          Gt  [kh, (Gre|Gim)] = f^T @ [Ci|Sq]-combo  (P3, data-stationary)
          enc [h, w]          = Fi^T @ Gt            (P4, const-stationary)
        src_loader(xt, hbl) fills xt [128, 256]; evict_fn(ap, hbl) consumes
        the final real [128, 256] block (an SBUF tile).
        """
        nc = self.nc
        C = self._consts
        DF = C["DF"]
        xp, zp, pp, ptp = pools

        def LT(nm, chunk):
            return DF[nm + ("_hi" if chunk == 0 else "_lo") + suffix32]

        # ---- load x [h, w]
        xt = [xp.tile([128, 256], dt, name="fft_xt") for _ in range(2)]
        for hbl in range(2):
            src_loader(xt[hbl], hbl)
        # ---- P1: Zt[wbl] = x^T @ [C|S]   (psum [w128, 512])
        Zt = []
        for wbl in range(2):
            pZ = pp.tile([128, 512], f32, name="fft_ps", bufs=5)
            for ch in range(2):
                nc.tensor.matmul(pZ[:], xt[ch][:, wbl * 128:(wbl + 1) * 128],
                                 LT("cCS", ch)[:], start=(ch == 0), stop=(ch == 1))
            zt = zp.tile([128, 512], dt, name="fft_zt")
            nc.vector.tensor_copy(zt[:], pZ[:])
            Zt.append(zt)
        # ---- P2: f[kwbl] [kw, (fre|fim)]; cross-terms accumulate in PSUM
        fsb = []
        for kwbl in range(2):
            pf = pp.tile([128, 512], f32, name="fft_ps", bufs=5)
            sl = slice(kwbl * 128, (kwbl + 1) * 128)
            for ch in range(2):
                nc.tensor.matmul(pf[:], LT("cC", ch)[:, sl], Zt[ch][:],
                                 start=(ch == 0), stop=False)
            for ch in range(2):
                nc.tensor.matmul(pf[:, 0:256], LT("cNS", ch)[:, sl],
                                 Zt[ch][:, 256:512], start=False, stop=False)
            for ch in range(2):
                nc.tensor.matmul(pf[:, 256:512], LT("cS", ch)[:, sl],
                                 Zt[ch][:, 0:256], start=False, stop=(ch == 1))
            ft = zp.tile([128, 512], dt, name="fft_ft")
            if mask:
                nc.vector.tensor_tensor(ft[:, 0:256], pf[:, 0:256],
                                        C["maskt"][kwbl][:], op=ALU.mult)
                nc.vector.tensor_tensor(ft[:, 256:512], pf[:, 256:512],
                                        C["maskt"][kwbl][:], op=ALU.mult)
            else:
                nc.vector.tensor_copy(ft[:], pf[:])
            fsb.append(ft)
        # ---- P3: Gt[khbl] [kh, (Gre|Gim)], data-stationary, PSUM cross-accum
        Gt = []
        for khbl in range(2):
            pG = pp.tile([128, 512], f32, name="fft_ps", bufs=5)
            sl = slice(khbl * 128, (khbl + 1) * 128)
            for ch in range(2):
                nc.tensor.matmul(pG[:, 0:256], fsb[ch][:, sl],
                                 LT("cCi", ch)[:], start=(ch == 0), stop=False)
            for ch in range(2):
                nc.tensor.matmul(pG[:, 0:256], fsb[ch][:, 256 + khbl * 128:256 + (khbl + 1) * 128],
                                 LT("cSq", ch)[:], start=False, stop=False)
            for ch in range(2):
                nc.tensor.matmul(pG[:, 256:512], fsb[ch][:, 256 + khbl * 128:256 + (khbl + 1) * 128],
                                 LT("cCi", ch)[:], start=(ch == 0), stop=False)
            for ch in range(2):
                nc.tensor.matmul(pG[:, 256:512], fsb[ch][:, sl],
                                 LT("cNSq", ch)[:], start=False, stop=(ch == 1))
            gt = zp.tile([128, 512], dt, name="fft_gt")
            nc.vector.tensor_copy(gt[:], pG[:])
            Gt.append(gt)
        # ---- P4: enc[hbl] = Ci^T@Gre + Sq^T@Gim (real)
        for hbl in range(2):
            pE = pp.tile([128, 256], f32, name="fft_pe", bufs=2)
            sl = slice(hbl * 128, (hbl + 1) * 128)
            for ch in range(2):
                nc.tensor.matmul(pE[:], LT("cCi", ch)[:, sl], Gt[ch][:, 0:256],
                                 start=(ch == 0), stop=False)
            for ch in range(2):
                nc.tensor.matmul(pE[:], LT("cSq", ch)[:, sl], Gt[ch][:, 256:512],
                                 start=False, stop=(ch == 1))
            esb = zp.tile([128, 256], f32, name="fft_esb")
            nc.vector.tensor_copy(esb[:], pE[:])
            evict_fn(esb, hbl)

    # ------------------------------------------------------------------
    def _conv0(self, X, tc):
        """conv0: lpn(3ch, negated/255-scaled) -> y0. K=9 via 3 stacked row-shifts."""
        nc = self.nc
        C = self._consts
        with ExitStack() as S:
            bp = S.enter_context(tc.tile_pool(name="band0", bufs=3))
            pp = S.enter_context(tc.tile_pool(name="psum0", bufs=3, space="PSUM"))
            ep = S.enter_context(tc.tile_pool(name="evict0", bufs=3))
            sp = S.enter_context(tc.tile_pool(name="stat0", bufs=1))
            ssum_cols = sp.tile([128, 64], f32, name="ssc0")
            sqsum_cols = sp.tile([128, 64], f32, name="sqc0")
            cidx = 0
            for bi in range(NBAND):
                r0 = bi * R
                band = bp.tile([18, (R + 2) * WP], f32r, name="bandt0")
                b3 = band[:].rearrange("p (r c) -> p r c", c=WP)
                i0 = 1 if bi == 0 else 0
                i1 = R + 1 if bi == NBAND - 1 else R + 2
                rl, rh = r0 - 1 + i0, r0 - 1 + i1
                nc.vector.memset(b3[0:3, :, 0:1].bitcast(f32), 0.0)
                nc.vector.memset(b3[0:3, :, 257:258].bitcast(f32), 0.0)
                if bi == 0:
                    nc.vector.memset(b3[0:3, 0:1, :].bitcast(f32), 0.0)
                if bi == NBAND - 1:
                    nc.vector.memset(b3[0:3, R + 1:R + 2, :].bitcast(f32), 0.0)
                nc.sync.dma_start(b3[0:3, i0:i1, 1:257],
                                  C["lpn_d"][:, rl:rh, :].bitcast(f32r))
                nc.sync.dma_start(b3[3:6, 0:R + 1, :], b3[0:3, 1:R + 2, :])
                nc.sync.dma_start(b3[6:9, 0:R, :], b3[0:3, 2:R + 2, :])
                nc.sync.dma_start(b3[9:18, 0:R, :], b3[0:9, 2:R + 2, :])
                for t in range(0, R, 4):
                    pt = pp.tile([128, 512], f32, name="pchunk0")
                    for dw in range(3):
                        nc.tensor.matmul(
                            pt[:], C["lhsT0"][dw][:],
                            b3[0:18, t:t + 2, dw:dw + 256],
                            start=(dw == 0), stop=(dw == 2))
                    ysb = ep.tile([128, 512], f32, name="ysb0")
                    nc.vector.tensor_scalar(ysb[:], pt[:], C["b128"][0][:, 0:1],
                                            0.0, op0=ALU.add, op1=ALU.add,
                                            accum_out=ssum_cols[:, cidx:cidx + 1])
                    scr = ep.tile([128, 512], f32, name="sqscr0")
                    nc.scalar.activation(scr[:], ysb[:], FT.Square,
                                         bias=C["zero128"][:, 0:1],
                                         accum_out=sqsum_cols[:, cidx:cidx + 1])
                    cidx += 1
                    q = r0 + t
                    nc.sync.dma_start(C["y"][0][:, q:q + 2, :],
                                      ysb[0:64, :].rearrange("p (r c) -> p r c", c=256))
                    nc.sync.dma_start(C["y"][0][:, q + 2:q + 4, :],
                                      ysb[64:128, :].rearrange("p (r c) -> p r c", c=256))
            return self._stats_finalize(tc, sp, 0, ssum_cols, sqsum_cols, cidx)

    # ------------------------------------------------------------------
    def _build_body(self, X, tc):
        nc = self.nc
        C = self._consts
        d = self.d

        # ============ low-pass filter (fp32), 3 channels ============
        with ExitStack() as S:
            xp = S.enter_context(tc.tile_pool(name="lp_x", bufs=4))
            zp = S.enter_context(tc.tile_pool(name="lp_z", bufs=6))
            pp = S.enter_context(tc.tile_pool(name="lp_ps", bufs=1, space="PSUM"))
            ptp = S.enter_context(tc.tile_pool(name="lp_tmp", bufs=4))
            ep = S.enter_context(tc.tile_pool(name="lp_ev", bufs=3))
            for c in range(3):
                def loader(xt, hbl, c=c):
                    nc.sync.dma_start(
                        xt[:], d["image"][c, hbl * 128:(hbl + 1) * 128, :])
                def evict(esb, hbl, c=c):
                    t1 = ep.tile([128, 256], f32, name="lp_t1")
                    nc.vector.tensor_scalar(t1[:], esb[:], 0.0, 255.0,
                                            op0=ALU.max, op1=ALU.min)
                    xi = ep.tile([128, 256], mybir.dt.int32, name="lp_xi")
                    nc.vector.tensor_copy(xi[:], t1[:])
                    xr = ep.tile([128, 256], f32, name="lp_xr")
                    nc.vector.tensor_copy(xr[:], xi[:])
                    gt = ep.tile([128, 256], f32, name="lp_gt")
                    nc.vector.tensor_tensor(gt[:], xr[:], t1[:], op=ALU.is_gt)
                    t2 = ep.tile([128, 256], f32, name="lp_t2")
                    nc.vector.tensor_tensor(t2[:], gt[:], xr[:], op=ALU.subtract)
                    nc.sync.dma_start(C["lpn_d"][c, hbl * 128:(hbl + 1) * 128, :], t2[:])
                self._fft_chain(tc, (xp, zp, pp, ptp),
                                src_loader=loader, dt=f32, suffix32="32",
                                mask=True, evict_fn=evict)

        # ============ conv0 ============
        sc0, sh0 = self._conv0(X, tc)

        # ============ conv1..3 ============
        sc, sh = sc0, sh0
        for k in range(1, 4):
            sc, sh = self._conv64(X, tc, k, C["y"][k - 1], C["y"][k], sc, sh)

        # bn3 scale/shift broadcast to 128 partitions: [128, 64] via transpose+K=1 matmul
        with ExitStack() as S:
            pp = S.enter_context(tc.tile_pool(name="bc_ps", bufs=2, space="PSUM"))
            bcp = S.enter_context(tc.tile_pool(name="bc", bufs=1))
            ones1 = bcp.tile([1, 128], f32, name="ones1")
            nc.vector.memset(ones1[:], 1.0)
            scale_bc = C["cp"].tile([128, 64], f32, name="scale_bc")
            shift_bc = C["cp"].tile([128, 64], f32, name="shift_bc")
            for vec, dst in [(sc, scale_bc), (sh, shift_bc)]:
                ptr = pp.tile([1, 64], f32, name="bc_tr")
                nc.tensor.transpose(ptr[:], vec[:], C["ident"][0:64, 0:64])
                row = bcp.tile([1, 64], f32, name="bc_row")
                nc.vector.tensor_copy(row[:], ptr[:])
                pbc = pp.tile([128, 64], f32, name="bc_mm")
                nc.tensor.matmul(pbc[:], ones1[:], row[:], start=True, stop=True)
                nc.vector.tensor_copy(dst[:], pbc[:])
            C["scale_bc"] = scale_bc
            C["shift_bc"] = shift_bc

        # ============ main fft chain (f32r), 64 channels ============
        with ExitStack() as S:
            xp = S.enter_context(tc.tile_pool(name="m_x", bufs=4))
            zp = S.enter_context(tc.tile_pool(name="m_z", bufs=6))
            pp = S.enter_context(tc.tile_pool(name="m_ps", bufs=1, space="PSUM"))
            ptp = S.enter_context(tc.tile_pool(name="m_tmp", bufs=2))
            for c in range(64):
                def loader(xt, hbl, c=c):
                    nc.sync.dma_start(
                        xt[:],
                        C["y"][3][c, hbl * 128:(hbl + 1) * 128, :].bitcast(f32r))
                    nc.scalar.activation(
                        xt[:], xt[:].bitcast(f32),
                        FT.Relu, bias=C["shift_bc"][:, c:c + 1],
                        scale=C["scale_bc"][:, c:c + 1])
                def evict(esb, hbl, c=c):
                    nc.sync.dma_start(
                        C["enc_d"][hbl * 128:(hbl + 1) * 128, c, :], esb[:])
                self._fft_chain(tc, (xp, zp, pp, ptp),
                                src_loader=loader, dt=f32r, suffix32="",
                                wm=(c == 0), evict_fn=evict)

        # ============ ac conv (97 -> 64) ============
        sc4, sh4 = self._conv64(X, tc, 4, None, C["y"][4], None, None)

        # ============ final 1x1 conv ============
        # rows packed: partitions 0:64 = ch, rows q..; 64:128 = ch, rows q+128..
        with ExitStack() as S:
            bp = S.enter_context(tc.tile_pool(name="fin_b", bufs=3))
            pp = S.enter_context(tc.tile_pool(name="fin_ps", bufs=4, space="PSUM"))
            ep = S.enter_context(tc.tile_pool(name="fin_ev", bufs=3))
            fb6 = C["cp"].tile([6, 1], f32, name="fb6")
            nc.sync.dma_start(fb6[0:3, :], d["fb"][:])
            nc.sync.dma_start(fb6[3:6, :], d["fb"][:])
            sc128 = C["cp"].tile([128, 1], f32, name="fin_sc128")
            sh128 = C["cp"].tile([128, 1], f32, name="fin_sh128")
            for half in range(2):
                nc.sync.dma_start(sc128[64 * half:64 * half + 64, :], sc4[:])
                nc.sync.dma_start(sh128[64 * half:64 * half + 64, :], sh4[:])
            FR = 16   # rows per chunk (per half)
            for q in range(0, 128, FR):
                xf = bp.tile([128, FR, 256], f32r, name="fin_x")
                nc.sync.dma_start(xf[0:64, :, :], C["y"][4][:, q:q + FR, :].bitcast(f32r))
                nc.sync.dma_start(xf[64:128, :, :],
                                  C["y"][4][:, 128 + q:128 + q + FR, :].bitcast(f32r))
                nc.scalar.activation(xf[:].rearrange("p r c -> p (r c)"),
                                     xf[:].rearrange("p r c -> p (r c)").bitcast(f32),
                                     FT.Relu, bias=sh128[:, 0:1], scale=sc128[:, 0:1])
                for rr in range(0, FR, 2):
                    pt = pp.tile([6, 512], f32, name="fin_p")
                    nc.tensor.matmul(pt[:], C["lhsT_fin"][:],
                                     xf[:, rr:rr + 2, :].rearrange("p r c -> p (r c)"),
                                     start=True, stop=True)
                    osb = ep.tile([6, 512], f32, name="fin_o")
                    nc.vector.tensor_scalar(osb[:], pt[:], fb6[:, 0:1], None, op0=ALU.add)
                    nc.sync.dma_start(d["out"][:, q + rr:q + rr + 2, :],
                                      osb[0:3, :].rearrange("p (r c) -> p r c", c=256))
                    nc.sync.dma_start(d["out"][:, 128 + q + rr:128 + q + rr + 2, :],
                                      osb[3:6, :].rearrange("p (r c) -> p r c", c=256))

        # debug outputs
        for nm, src, shp in [("lpn", C["lpn_d"], (3, H, W)),
                             ("y0", C["y"][0], (64, H, W)),
                             ("y1", C["y"][1], (64, H, W)),
                             ("y2", C["y"][2], (64, H, W)),
                             ("y3", C["y"][3], (64, H, W)),
                             ("enc", C["enc_d"], (H, 64, W)),
                             ("y4", C["y"][4], (64, H, W))]:
            ap = self.maybe_debug(nm, shp)
            if ap is not None:
                nc.sync.dma_start(ap[:], src[:])



# ======================================================================
# harness entry point: full inputs in, full outputs out (8 cores SPMD)
# ======================================================================
from concourse.bass_utils import run_bass_kernel_spmd

_ENC = None

def _get_enc():
    global _ENC
    if _ENC is None:
        e = Enc(n_cores=8)
        e.build()
        _ENC = e
    return _ENC

def make_in_maps(inputs):
    consts = host_constants()
    g = lambda k: np.ascontiguousarray(np.asarray(inputs[k], dtype=np.float32))
    image, message = g("image"), g("message")
    shared = dict(
        w0=g("w0"), b0=g("b0").reshape(64, 1), g0=g("g0").reshape(64, 1),
        be0=g("be0").reshape(64, 1), ws=g("ws"), bs=g("bs").reshape(3, 64, 1),
        gs=g("gs").reshape(3, 64, 1), bes=g("bes").reshape(3, 64, 1),
        acw=g("acw"), acb=g("acb").reshape(64, 1), acg=g("acg").reshape(64, 1),
        acbe=g("acbe").reshape(64, 1), fw=np.ascontiguousarray(g("fw")[:, :, 0, 0]),
        fb=g("fb").reshape(3, 1), **consts)
    return [dict(image=np.ascontiguousarray(image[i]),
                 message=np.ascontiguousarray(message[i].reshape(MSG, 1)),
                 **shared) for i in range(8)]

def kernel(**inputs):
    e = _get_enc()
    in_maps = make_in_maps(inputs)
    res = run_bass_kernel_spmd(e.nc, in_maps, core_ids=list(range(8)))
    out = np.stack([res.results[i]["out"] for i in range(8)], axis=0)
    return np.ascontiguousarray(out.astype(np.float32))



# revision 22
# speedup vs baseline: 1.3190x; 1.3190x over previous
"""Encoder kernel: full Bass/Tile program for nn_Encoder (watermark encoder)
on 8 TRN2 cores, data-parallel over batch (1 image per core).

Key structure (v2):
  - The tree-ring watermark only touches fft channel 0, so ifft(fft(x))=x
    exactly for channels 1..63: only channel 0 runs the fft->edit->ifft
    chain; the other 63 "enc" channels are BN-ReLU(y3) read directly.
  - All conv activations + weights are fp16 (PSUM stays fp32); activations
    are stored in DRAM with 258-wide zero-padded rows so band loads /
    evictions are large contiguous descriptors.
  - conv0 and the ac-conv's img part fold the 3x3 dw taps into the K dim
    (column-shifted copies in SBUF), one matmul per 4-row chunk.
  - The 30 constant message channels of the ac conv are folded into a
    single all-ones virtual channel with kernel W~[o,a,b] = sum_i msg_i *
    acw[o,i,a,b] (exact, including zero-pad borders).
  - All weight lhsT tiles are precomputed on the host (numpy) in fp16.
  - Low-pass fft chain runs in f32r.

Per-core DRAM scratch layouts:
  y0..y4:  [64, 256, 258] fp16   (row-padded: col 0 and 257 are zero)
  lpn_d:   [3, 256, 258]  fp16   (holds floor(clip(low,0,255)); w0 is /255)
  enc0_d:  [256, 258]     fp16   (watermarked enc channel 0)
"""
import numpy as np
import concourse.bass as bass
import concourse.tile as tile
from concourse import bacc, mybir
from contextlib import ExitStack

f32 = mybir.dt.float32
f32r = mybir.dt.float32r
f16 = mybir.dt.float16
FT = mybir.ActivationFunctionType
ALU = mybir.AluOpType

H = W = 256
HW = H * W
CH = 64
MSG = 30
WP = W + 2          # padded row length 258
R = 32              # conv band rows
NBAND = H // R      # 8
BANDL = (R + 2) * WP   # flat band length 8772


# ---------------------------------------------------------------- host consts
def host_constants():
    j = np.arange(256)
    ang = 2.0 * np.pi * np.outer(j, j) / 256.0
    C = np.cos(ang).astype(np.float32)
    S = (-np.sin(ang)).astype(np.float32)      # F = C + iS
    Ci = (C / 256.0).astype(np.float32)
    Sq = (S / 256.0).astype(np.float32)
    consts = {
        "cC": C, "cS": S, "cNS": -S, "cCi": Ci, "cSq": Sq, "cNSq": -Sq,
        "cCS": np.hstack([C, S]).astype(np.float32),
    }
    # low-pass mask, ifftshifted
    yy = np.arange(H, dtype=np.float32) - H // 2
    xx = np.arange(W, dtype=np.float32) - W // 2
    m = ((yy[:, None] ** 2 + xx[None, :] ** 2) <= float(50 * 50)).astype(np.float32)
    consts["cMask"] = np.fft.ifftshift(m).astype(np.float32)
    return consts


def _wm_grid(msg):
    """wmv[a,b] = message value at (kw=124+a, kh=124+b); kap = indicator."""
    rr, cc = [], []
    idx = 0
    for i in range(-4, 5):
        for j2 in range(-4, 5):
            if idx >= MSG:
                break
            if (i * i + j2 * j2) ** 0.5 <= 4:
                rr.append(128 + i); cc.append(128 + j2); idx += 1
    wmv = np.zeros((9, 9), np.float32)
    kap = np.zeros((9, 9), np.float32)
    for k in range(MSG):
        wmv[cc[k] - 124, rr[k] - 124] = msg[k]
        kap[cc[k] - 124, rr[k] - 124] = 1.0
    return wmv, kap


def host_weights(w0, ws, acw, fw, message):
    """Precompute all fp16 lhsT tiles + per-core wm edit tiles (numpy)."""
    out = {}
    # conv1..3 + ac enc pairs: pA/pB [128,128], cx [64,128] per (layer, dw)
    def pair_set(wt):  # wt [64o, 64i, 3, 3]
        pAs, pBs, cxs = [], [], []
        for dw in range(3):
            tp = [np.ascontiguousarray(wt[:, :, a, dw].T) for a in range(3)]  # [i,o]
            pA = np.zeros((128, 128), np.float32)
            pB = np.zeros((128, 128), np.float32)
            cx = np.zeros((64, 128), np.float32)
            pA[0:64, 0:64] = tp[0]; pA[64:128, 0:64] = tp[1]
            pB[0:64, 64:128] = tp[1]; pB[64:128, 64:128] = tp[2]
            cx[:, 0:64] = tp[2]; cx[:, 64:128] = tp[0]
            pAs.append(pA); pBs.append(pB); cxs.append(cx)
        return pAs, pBs, cxs

    for k in range(3):
        pAs, pBs, cxs = pair_set(ws[k])
        out[f"lA{k}"] = np.stack(pAs).astype(np.float16)
        out[f"lB{k}"] = np.stack(pBs).astype(np.float16)
        out[f"lX{k}"] = np.stack(cxs).astype(np.float16)
    pAs, pBs, cxs = pair_set(acw[:, 30:94])
    out["lAa"] = np.stack(pAs).astype(np.float16)
    out["lBa"] = np.stack(pBs).astype(np.float16)
    out["lXa"] = np.stack(cxs).astype(np.float16)

    # conv0: [54, 128]; p = (dh*3+dw)*3+ci
    l0 = np.zeros((54, 128), np.float32)
    for dh in range(3):
        for dw in range(3):
            for ci in range(3):
                p = (dh * 3 + dw) * 3 + ci
                l0[p, 0:64] = w0[:, ci, dh, dw] / 255.0
                l0[27 + p, 64:128] = w0[:, ci, dh, dw] / 255.0
    out["l0"] = l0.astype(np.float16)

    # ac imgones: [72, 128]; p = (dh*3+dw)*4+ci; ci=3 -> msg-folded kernel
    wtil = np.einsum("i,oiab->oab", message.astype(np.float64),
                     acw[:, 0:30].astype(np.float64)).astype(np.float32)
    li = np.zeros((72, 128), np.float32)
    for dh in range(3):
        for dw in range(3):
            for ci in range(4):
                p = (dh * 3 + dw) * 4 + ci
                v = acw[:, 94 + ci, dh, dw] if ci < 3 else wtil[:, dh, dw]
                li[p, 0:64] = v
                li[36 + p, 64:128] = v
    out["li"] = li.astype(np.float16)

    # final 1x1: [128, 6]
    lf = np.zeros((128, 6), np.float32)
    lf[0:64, 0:3] = fw.T
    lf[64:128, 3:6] = fw.T
    out["lf"] = lf.astype(np.float16)

    # wm edit tiles [128, 9] f32
    wmv, kap = _wm_grid(message)
    wm0 = np.zeros((128, 9), np.float32); wm1 = np.zeros((128, 9), np.float32)
    om0 = np.ones((128, 9), np.float32); om1 = np.ones((128, 9), np.float32)
    wm0[124:128, :] = wmv[0:4, :]; wm1[0:5, :] = wmv[4:9, :]
    om0[124:128, :] = 1.0 - kap[0:4, :]; om1[0:5, :] = 1.0 - kap[4:9, :]
    out["wm0"] = wm0; out["wm1"] = wm1; out["om0"] = om0; out["om1"] = om1
    return out


# ---------------------------------------------------------------- builders
class Enc:
    def __init__(self, n_cores=8, debug_outs=(), stop_after=None):
        self.n_cores = n_cores
        self.ntot = float(n_cores * HW)
        self.debug_outs = debug_outs
        self.stop_after = stop_after  # debug: truncate body after phase N
        self.no_coll = False          # debug: replace AllReduce with local copy
        nc = bacc.Bacc("TRN2", target_bir_lowering=False, debug=False,
                       num_devices=n_cores)
        self.nc = nc
        d = {}
        d["image"] = nc.dram_tensor("image", (3, H, W), f32, kind="ExternalInput").ap()
        # 4 channels: img0..2 + all-ones (for the msg-folded virtual channel)
        d["img16"] = nc.dram_tensor("img16", (4, H, WP), f16, kind="ExternalInput").ap()
        for k in range(3):
            for nm in ("lA", "lB"):
                d[f"{nm}{k}"] = nc.dram_tensor(f"{nm}{k}", (3, 128, 128), f16,
                                               kind="ExternalInput").ap()
            d[f"lX{k}"] = nc.dram_tensor(f"lX{k}", (3, 64, 128), f16,
                                         kind="ExternalInput").ap()
        d["lAa"] = nc.dram_tensor("lAa", (3, 128, 128), f16, kind="ExternalInput").ap()
        d["lBa"] = nc.dram_tensor("lBa", (3, 128, 128), f16, kind="ExternalInput").ap()
        d["lXa"] = nc.dram_tensor("lXa", (3, 64, 128), f16, kind="ExternalInput").ap()
        d["l0"] = nc.dram_tensor("l0", (54, 128), f16, kind="ExternalInput").ap()
        d["li"] = nc.dram_tensor("li", (72, 128), f16, kind="ExternalInput").ap()
        d["lf"] = nc.dram_tensor("lf", (128, 6), f16, kind="ExternalInput").ap()
        for nm in ("wm0", "wm1", "om0", "om1"):
            d[nm] = nc.dram_tensor(nm, (128, 9), f32, kind="ExternalInput").ap()
        d["b0"] = nc.dram_tensor("b0", (64, 1), f32, kind="ExternalInput").ap()
        d["g0"] = nc.dram_tensor("g0", (64, 1), f32, kind="ExternalInput").ap()
        d["be0"] = nc.dram_tensor("be0", (64, 1), f32, kind="ExternalInput").ap()
        d["bs"] = nc.dram_tensor("bs", (3, 64, 1), f32, kind="ExternalInput").ap()
        d["gs"] = nc.dram_tensor("gs", (3, 64, 1), f32, kind="ExternalInput").ap()
        d["bes"] = nc.dram_tensor("bes", (3, 64, 1), f32, kind="ExternalInput").ap()
        d["acb"] = nc.dram_tensor("acb", (64, 1), f32, kind="ExternalInput").ap()
        d["acg"] = nc.dram_tensor("acg", (64, 1), f32, kind="ExternalInput").ap()
        d["acbe"] = nc.dram_tensor("acbe", (64, 1), f32, kind="ExternalInput").ap()
        d["fb"] = nc.dram_tensor("fb", (3, 1), f32, kind="ExternalInput").ap()
        for k, shp in [("cC", (256, 256)), ("cS", (256, 256)), ("cNS", (256, 256)),
                       ("cCi", (256, 256)), ("cSq", (256, 256)), ("cNSq", (256, 256)),
                       ("cCS", (256, 512)), ("cMask", (256, 256))]:
            d[k] = nc.dram_tensor(k, shp, f32, kind="ExternalInput").ap()
        d["out"] = nc.dram_tensor("out", (3, H, W), f32, kind="ExternalOutput").ap()
        self.d = d
        self.dbg = {}

    def maybe_debug(self, name, shape, dtype=f32):
        if name in self.debug_outs:
            self.dbg[name] = self.nc.dram_tensor(
                "dbg_" + name, shape, dtype, kind="ExternalOutput").ap()
            return self.dbg[name]
        return None

    # ------------------------------------------------------------------
    def build(self):
        nc, d = self.nc, self.d
        with tile.TileContext(nc) as tc, ExitStack() as X:
            cp = X.enter_context(tc.tile_pool(name="consts", bufs=1))
            dp = X.enter_context(tc.tile_pool(name="dram", bufs=1, space="DRAM"))

            # ---------------- DRAM scratch
            y = [dp.tile([64, H, WP], f16, name=f"yact{k}") for k in range(5)]
            lpn_d = dp.tile([3, H, WP], f16, name="lpn_d")
            enc0_d = dp.tile([1, H, WP], f16, name="enc0_d")
            cl_in = [dp.tile([64, 2], f32, name=f"clin{k}") for k in range(5)]
            cl_out = [dp.tile([64, 2], f32, name=f"clout{k}", addr_space="Shared")
                      for k in range(5)]

            # ---------------- constants into SBUF
            def cload(name, src, shape, dtype):
                t = cp.tile(shape, dtype, name=name)
                nc.sync.dma_start(t[:], src if dtype == f32 else src.bitcast(dtype))
                return t
            DF = {}
            for nm, wdt in [("cC", 256), ("cS", 256), ("cNS", 256), ("cCi", 256),
                            ("cSq", 256), ("cNSq", 256), ("cCS", 512)]:
                DF[nm + "_hi"] = cload(nm + "_hi", d[nm][0:128, :], [128, wdt], f32r)
                DF[nm + "_lo"] = cload(nm + "_lo", d[nm][128:256, :], [128, wdt], f32r)
            maskt = [cload(f"maskt{i}", d["cMask"][i * 128:(i + 1) * 128, :],
                           [128, 256], f32) for i in range(2)]
            wm_al = [cload("wm_al0", d["wm0"][:], [128, 9], f32),
                     cload("wm_al1", d["wm1"][:], [128, 9], f32)]
            onemk_al = [cload("om_al0", d["om0"][:], [128, 9], f32),
                        cload("om_al1", d["om1"][:], [128, 9], f32)]
            zero128 = cp.tile([128, 1], f32, name="zero128")
            nc.vector.memset(zero128[:], 0.0)
            eps64 = cp.tile([64, 1], f32, name="eps64")
            nc.vector.memset(eps64[:], 1e-5)
            ones1 = cp.tile([1, 128], f32, name="ones1")
            nc.vector.memset(ones1[:], 1.0)

            # weight lhsT tiles (host-precomputed, fp16)
            WT = {}
            for k in range(4):
                sfx = str(k) if k < 3 else "a"
                WT[f"pA{k}"] = [cload(f"pA{k}{dw}", d[f"lA{sfx}"][dw], [128, 128], f16)
                                for dw in range(3)]
                WT[f"pB{k}"] = [cload(f"pB{k}{dw}", d[f"lB{sfx}"][dw], [128, 128], f16)
                                for dw in range(3)]
                WT[f"cx{k}"] = [cload(f"cx{k}{dw}", d[f"lX{sfx}"][dw], [64, 128], f16)
                                for dw in range(3)]
            lhsT0 = cload("lhsT0", d["l0"][:], [54, 128], f16)
            lhsT_io = cload("lhsT_io", d["li"][:], [72, 128], f16)
            lhsT_fin = cload("lhsT_fin", d["lf"][:], [128, 6], f16)

            # bn params / biases
            def vload(name, src):
                t = cp.tile([64, 1], f32, name=name)
                nc.sync.dma_start(t[:], src)
                return t
            g_t = [vload("g_t0", d["g0"][:])] + \
                  [vload(f"g_t{k+1}", d["gs"][k]) for k in range(3)] + \
                  [vload("g_t4", d["acg"][:])]
            be_t = [vload("be_t0", d["be0"][:])] + \
                   [vload(f"be_t{k+1}", d["bes"][k]) for k in range(3)] + \
                   [vload("be_t4", d["acbe"][:])]
            b128 = []
            for k, src in enumerate([d["b0"], d["bs"][0], d["bs"][1], d["bs"][2],
                                     d["acb"]]):
                t = cp.tile([128, 1], f32, name=f"b128_{k}")
                nc.sync.dma_start(t[0:64, :], src)
                nc.sync.dma_start(t[64:128, :], src)
                b128.append(t)
            fb6 = cp.tile([6, 1], f32, name="fb6")
            nc.sync.dma_start(fb6[0:3, :], d["fb"][:])
            nc.sync.dma_start(fb6[3:6, :], d["fb"][:])

            self._consts = dict(DF=DF, maskt=maskt, wm_al=wm_al, onemk_al=onemk_al,
                                zero128=zero128, eps64=eps64, ones1=ones1,
                                WT=WT, lhsT0=lhsT0, lhsT_io=lhsT_io,
                                lhsT_fin=lhsT_fin,
                                g_t=g_t, be_t=be_t, b128=b128, fb6=fb6, cp=cp,
                                y=y, lpn_d=lpn_d, enc0_d=enc0_d,
                                cl_in=cl_in, cl_out=cl_out)
            self._build_body(X, tc)
        self.nc.compile()

    # ------------------------------------------------------------------
    def _stats_finalize(self, tc, pool, layer, ssum_cols, sqsum_cols, ncols):
        """Reduce per-chunk stat columns, AllReduce, return (scale, shift) [64,1]."""
        nc = self.nc
        C = self._consts
        red = pool.tile([128, 2], f32, name=f"red{layer}")
        nc.vector.tensor_reduce(red[:, 0:1], ssum_cols[:, 0:ncols], axis=mybir.AxisListType.X, op=ALU.add)
        nc.vector.tensor_reduce(red[:, 1:2], sqsum_cols[:, 0:ncols], axis=mybir.AxisListType.X, op=ALU.add)
        upper = pool.tile([64, 2], f32, name=f"upper{layer}")
        nc.sync.dma_start(upper[:], red[64:128, :])
        stats = pool.tile([64, 2], f32, name=f"stats{layer}")
        nc.vector.tensor_add(stats[:], red[0:64, :], upper[:])
        nc.sync.dma_start(C["cl_in"][layer][:], stats[:])
        if self.no_coll:
            nc.sync.dma_start(C["cl_out"][layer][:], C["cl_in"][layer][:])
        else:
            nc.gpsimd.collective_compute(
                "AllReduce", ALU.add,
                replica_groups=[list(range(self.n_cores))],
                ins=[C["cl_in"][layer].opt()], outs=[C["cl_out"][layer].opt()])
        sr = pool.tile([64, 2], f32, name=f"sr{layer}")
        nc.sync.dma_start(sr[:], C["cl_out"][layer][:])
        mean = pool.tile([64, 1], f32, name=f"mean{layer}")
        nc.vector.tensor_scalar_mul(mean[:], sr[:, 0:1], 1.0 / self.ntot)
        ms = pool.tile([64, 1], f32, name=f"ms{layer}")
        nc.vector.tensor_scalar_mul(ms[:], sr[:, 1:2], 1.0 / self.ntot)
        msq = pool.tile([64, 1], f32, name=f"msq{layer}")
        nc.vector.tensor_scalar(msq[:], mean[:], mean[:, 0:1], None, op0=ALU.mult)
        var = pool.tile([64, 1], f32, name=f"var{layer}")
        nc.vector.tensor_scalar(var[:], ms[:], msq[:, 0:1], None, op0=ALU.subtract)
        std = pool.tile([64, 1], f32, name=f"std{layer}")
        nc.scalar.activation(std[:], var[:], FT.Sqrt, bias=C["eps64"][:, 0:1], scale=1.0)
        istd = pool.tile([64, 1], f32, name=f"istd{layer}")
        nc.vector.reciprocal(istd[:], std[:])
        # scale/shift outlive this layer's pools: allocate from the consts pool
        scale = C["cp"].tile([64, 1], f32, name=f"scale{layer}")
        nc.vector.tensor_tensor(scale[:], C["g_t"][layer][:], istd[:], op=ALU.mult)
        prod = pool.tile([64, 1], f32, name=f"prod{layer}")
        nc.vector.tensor_tensor(prod[:], mean[:], scale[:], op=ALU.mult)
        shift = C["cp"].tile([64, 1], f32, name=f"shift{layer}")
        nc.vector.scalar_tensor_tensor(shift[:], prod[:], -1.0, C["be_t"][layer][:],
                                       op0=ALU.mult, op1=ALU.add)
        return scale, shift

    # ------------------------------------------------------------------
    def _evict_chunk(self, layer, pp_tile, ep, dst, q, ssum_cols, sqsum_cols, cidx,
                     nbufs=3):
        """Evict psum chunk [128,512] -> fp16 padded dst rows q..q+3 + stats."""
        nc = self.nc
        C = self._consts
        ysb = ep.tile([128, 2, WP], f16, name=f"ysb{layer}")
        nc.vector.memset(ysb[:, :, 0:1], 0.0)
        nc.vector.memset(ysb[:, :, 257:258], 0.0)
        nc.vector.tensor_scalar(
            ysb[:, :, 1:257],
            pp_tile[:].rearrange("p (r c) -> p r c", c=256),
            C["b128"][layer][:, 0:1], 0.0, op0=ALU.add, op1=ALU.add,
            accum_out=ssum_cols[:, cidx:cidx + 1])
        scr = ep.tile([128, 2, 256], f16, name=f"sqscr{layer}")
        nc.scalar.activation(scr[:], ysb[:, :, 1:257], FT.Square,
                             bias=C["zero128"][:, 0:1],
                             accum_out=sqsum_cols[:, cidx:cidx + 1])
        nc.sync.dma_start(dst[:, q:q + 2, :], ysb[0:64])
        nc.sync.dma_start(dst[:, q + 2:q + 4, :], ysb[64:128])

    # ------------------------------------------------------------------
    def _conv64(self, X, tc, layer, src, dst, scale, shift, enc0=None):
        """conv layers 1..3 (64->64) and ac enc part (layer 4).

        If enc0 is given (ac conv), partition 0 of the band is overwritten
        with enc0 rows (post-wm channel) after the BN-ReLU activation, and
        the img+ones band contributes 1 extra matmul per chunk.
        """
        nc = self.nc
        C = self._consts
        is_ac = (layer == 4)
        with ExitStack() as S:
            bp = S.enter_context(tc.tile_pool(name=f"band{layer}", bufs=3))
            if is_ac:
                ip = S.enter_context(tc.tile_pool(name="ioband", bufs=3))
            pp = S.enter_context(tc.tile_pool(name=f"psum{layer}", bufs=4, space="PSUM"))
            ep = S.enter_context(tc.tile_pool(name=f"evict{layer}", bufs=3))
            sp = S.enter_context(tc.tile_pool(name=f"stat{layer}", bufs=1))
            ssum_cols = sp.tile([128, 64], f32, name=f"ssc{layer}")
            sqsum_cols = sp.tile([128, 64], f32, name=f"sqc{layer}")
            cidx = 0
            for bi in range(NBAND):
                r0 = bi * R
                band = bp.tile([128, BANDL], f16, name=f"bandt{layer}")
                b3 = band[:].rearrange("p (r c) -> p r c", c=WP)
                i0 = 1 if bi == 0 else 0
                i1 = R + 1 if bi == NBAND - 1 else R + 2
                rl, rh = r0 - 1 + i0, r0 - 1 + i1
                if bi == 0:
                    nc.vector.memset(band[0:64, 0:WP], 0.0)
                if bi == NBAND - 1:
                    nc.vector.memset(band[0:64, (R + 1) * WP:BANDL], 0.0)
                nc.sync.dma_start(b3[0:64, i0:i1, :], src[:, rl:rh, :])
                # fused BN+ReLU on interior rows (pads stay zero)
                nc.scalar.activation(b3[0:64, i0:i1, 1:257],
                                     b3[0:64, i0:i1, 1:257],
                                     FT.Relu, bias=shift[:, 0:1], scale=scale[:, 0:1])
                if is_ac:
                    # overwrite channel 0 with the watermarked enc channel
                    nc.sync.dma_start(b3[0:1, i0:i1, :], enc0[0:1, rl:rh, :])
                # dup: partitions 64:128 = band shifted one row down (flat copy)
                nc.sync.dma_start(band[64:128, 0:(R + 1) * WP],
                                  band[0:64, WP:BANDL])

                if is_ac:
                    # img + ones band [72, BANDL]: base at (dh,dw)=(0,1) -> p 4:8
                    ib = ip.tile([72, BANDL], f16, name="ibandt")
                    i3 = ib[:].rearrange("p (r c) -> p r c", c=WP)
                    if bi == 0:
                        nc.vector.memset(ib[0:8, 0:WP], 0.0)
                    if bi == NBAND - 1:
                        nc.vector.memset(ib[0:8, (R + 1) * WP:BANDL], 0.0)
                    nc.sync.dma_start(i3[4:8, i0:i1, :], C["img16_src"][:, rl:rh, :])
                    for dh in range(3):
                        for dw in range(3):
                            if (dh, dw) == (0, 1):
                                continue
                            delta = dh * WP + dw - 1
                            p = (dh * 3 + dw) * 4
                            a = max(0, -delta)
                            b = BANDL - max(0, delta)
                            nc.sync.dma_start(ib[p:p + 4, a:b],
                                              ib[4:8, a + delta:b + delta])
                    nc.sync.dma_start(ib[36:72, 0:R * WP], ib[0:36, 2 * WP:BANDL])

                for t in range(0, R, 4):
                    pt = pp.tile([128, 512], f32, name=f"pchunk{layer}")
                    nmm = 10 if is_ac else 9
                    mm = 0
                    for dw in range(3):
                        nc.tensor.matmul(
                            pt[:], C["WT"][f"pA{layer - 1}"][dw][:],
                            b3[0:128, t:t + 2, dw:dw + 256],
                            start=(mm == 0), stop=(mm == nmm - 1))
                        mm += 1
                        nc.tensor.matmul(
                            pt[:], C["WT"][f"pB{layer - 1}"][dw][:],
                            b3[0:128, t + 3:t + 5, dw:dw + 256],
                            start=(mm == 0), stop=(mm == nmm - 1))
                        mm += 1
                        nc.tensor.matmul(
                            pt[:], C["WT"][f"cx{layer - 1}"][dw][:],
                            b3[0:64, t + 2:t + 4, dw:dw + 256],
                            start=(mm == 0), stop=(mm == nmm - 1))
                        mm += 1
                    if is_ac:
                        nc.tensor.matmul(
                            pt[:], C["lhsT_io"][:],
                            i3[0:72, t:t + 2, 1:257],
                            start=False, stop=True)
                        mm += 1
                    self._evict_chunk(layer, pt, ep, dst, r0 + t,
                                      ssum_cols, sqsum_cols, cidx)
                    cidx += 1
            return self._stats_finalize(tc, sp, layer, ssum_cols, sqsum_cols, cidx)

    # ------------------------------------------------------------------
    def _conv0(self, X, tc):
        """conv0: lpn (3ch fp16, floor-valued) -> y0; dw folded into K=54."""
        nc = self.nc
        C = self._consts
        with ExitStack() as S:
            bp = S.enter_context(tc.tile_pool(name="band0", bufs=3))
            pp = S.enter_context(tc.tile_pool(name="psum0", bufs=4, space="PSUM"))
            ep = S.enter_context(tc.tile_pool(name="evict0", bufs=3))
            sp = S.enter_context(tc.tile_pool(name="stat0", bufs=1))
            ssum_cols = sp.tile([128, 64], f32, name="ssc0")
            sqsum_cols = sp.tile([128, 64], f32, name="sqc0")
            cidx = 0
            for bi in range(NBAND):
                r0 = bi * R
                band = bp.tile([54, BANDL], f16, name="bandt0")
                b3 = band[:].rearrange("p (r c) -> p r c", c=WP)
                i0 = 1 if bi == 0 else 0
                i1 = R + 1 if bi == NBAND - 1 else R + 2
                rl, rh = r0 - 1 + i0, r0 - 1 + i1
                # base channels at (dh,dw)=(0,1) -> partitions 3:6
                if bi == 0:
                    nc.vector.memset(band[0:6, 0:WP], 0.0)
                if bi == NBAND - 1:
                    nc.vector.memset(band[0:6, (R + 1) * WP:BANDL], 0.0)
                nc.sync.dma_start(b3[3:6, i0:i1, :], C["lpn_d"][:, rl:rh, :])
                for dh in range(3):
                    for dw in range(3):
                        if (dh, dw) == (0, 1):
                            continue
                        delta = dh * WP + dw - 1
                        p = (dh * 3 + dw) * 3
                        a = max(0, -delta)
                        b = BANDL - max(0, delta)
                        nc.sync.dma_start(band[p:p + 3, a:b],
                                          band[3:6, a + delta:b + delta])
                nc.sync.dma_start(band[27:54, 0:R * WP], band[0:27, 2 * WP:BANDL])
                for t in range(0, R, 4):
                    pt = pp.tile([128, 512], f32, name="pchunk0")
                    nc.tensor.matmul(pt[:], C["lhsT0"][:],
                                     b3[0:54, t:t + 2, 1:257],
                                     start=True, stop=True)
                    self._evict_chunk(0, pt, ep, C["y"][0], r0 + t,
                                      ssum_cols, sqsum_cols, cidx)
                    cidx += 1
            return self._stats_finalize(tc, sp, 0, ssum_cols, sqsum_cols, cidx)

    # ------------------------------------------------------------------
    def _fft_chain(self, tc, pools, *, src_loader, wm=False, mask=False,
                   evict_fn=None):
        """Transpose-free fused fft2 -> edit -> ifft2 for ONE channel (f32r).

        Layouts (per channel):
          x   [h, w]                     (xt tiles, 2 h-blocks)
          Zt  [w, (Zre|Zim)]  = x^T @ [C|S]          (P1, data-stationary)
          f   [kw, (fre|fim)] = Fw^T @ Zt            (P2, const-stationary)
          Gt  [kh, (Gre|Gim)] = f^T @ [Ci|Sq]-combo  (P3, data-stationary)
          enc [h, w]          = Fi^T @ Gt            (P4, const-stationary)
        """
        nc = self.nc
        C = self._consts
        DF = C["DF"]
        xp, zp, pp = pools
        dt = f32r

        def LT(nm, chunk):
            return DF[nm + ("_hi" if chunk == 0 else "_lo")]

        xt = [xp.tile([128, 256], dt, name="fft_xt") for _ in range(2)]
        for hbl in range(2):
            src_loader(xt[hbl], hbl)
        # ---- P1
        Zt = []
        for wbl in range(2):
            pZ = pp.tile([128, 512], f32, name="fft_ps", bufs=5)
            for ch in range(2):
                nc.tensor.matmul(pZ[:], xt[ch][:, wbl * 128:(wbl + 1) * 128],
                                 LT("cCS", ch)[:], start=(ch == 0), stop=(ch == 1))
            zt = zp.tile([128, 512], dt, name="fft_zt")
            nc.scalar.activation(zt[:], pZ[:], FT.Copy, bias=0.0, scale=1.0)
            Zt.append(zt)
        # ---- P2 (+ mask or wm edit)
        fsb = []
        for kwbl in range(2):
            pf = pp.tile([128, 512], f32, name="fft_ps", bufs=5)
            sl = slice(kwbl * 128, (kwbl + 1) * 128)
            for ch in range(2):
                nc.tensor.matmul(pf[:], LT("cC", ch)[:, sl], Zt[ch][:],
                                 start=(ch == 0), stop=False)
            for ch in range(2):
                nc.tensor.matmul(pf[:, 0:256], LT("cNS", ch)[:, sl],
                                 Zt[ch][:, 256:512], start=False, stop=False)
            for ch in range(2):
                nc.tensor.matmul(pf[:, 256:512], LT("cS", ch)[:, sl],
                                 Zt[ch][:, 0:256], start=False, stop=(ch == 1))
            ft = zp.tile([128, 512], dt, name="fft_ft")
            if mask:
                nc.vector.tensor_tensor(ft[:, 0:256], pf[:, 0:256],
                                        C["maskt"][kwbl][:], op=ALU.mult)
                nc.vector.tensor_tensor(ft[:, 256:512], pf[:, 256:512],
                                        C["maskt"][kwbl][:], op=ALU.mult)
            else:
                nc.vector.tensor_copy(ft[:], pf[:])
            if wm:
                fv = ft[:].bitcast(f32)
                for base in (124, 256 + 124):
                    nc.vector.tensor_tensor(ft[:, base:base + 9],
                                            fv[:, base:base + 9],
                                            C["onemk_al"][kwbl][:], op=ALU.mult)
                    nc.vector.tensor_tensor(ft[:, base:base + 9],
                                            fv[:, base:base + 9],
                                            C["wm_al"][kwbl][:], op=ALU.add)
            fsb.append(ft)
        # ---- P3
        Gt = []
        for khbl in range(2):
            pG = pp.tile([128, 512], f32, name="fft_ps", bufs=5)
            sl = slice(khbl * 128, (khbl + 1) * 128)
            for ch in range(2):
                nc.tensor.matmul(pG[:, 0:256], fsb[ch][:, sl],
                                 LT("cCi", ch)[:], start=(ch == 0), stop=False)
            for ch in range(2):
                nc.tensor.matmul(pG[:, 0:256], fsb[ch][:, 256 + khbl * 128:256 + (khbl + 1) * 128],
                                 LT("cSq", ch)[:], start=False, stop=False)
            for ch in range(2):
                # second start in the same psum bank: legal on HW (per-element
                # has_written), but the sim's 2KB zero-region bookkeeping
                # would flag it — skip that check.
                nc.tensor.matmul(pG[:, 256:512], fsb[ch][:, 256 + khbl * 128:256 + (khbl + 1) * 128],
                                 LT("cCi", ch)[:], start=(ch == 0), stop=False,
                                 skip_group_check=(ch == 0))
            for ch in range(2):
                nc.tensor.matmul(pG[:, 256:512], fsb[ch][:, sl],
                                 LT("cNSq", ch)[:], start=False, stop=(ch == 1))
            gt = zp.tile([128, 512], dt, name="fft_gt")
            nc.scalar.activation(gt[:], pG[:], FT.Copy, bias=0.0, scale=1.0)
            Gt.append(gt)
        # ---- P4
        for hbl in range(2):
            pE = pp.tile([128, 256], f32, name="fft_pe", bufs=2)
            sl = slice(hbl * 128, (hbl + 1) * 128)
            for ch in range(2):
                nc.tensor.matmul(pE[:], LT("cCi", ch)[:, sl], Gt[ch][:, 0:256],
                                 start=(ch == 0), stop=False)
            for ch in range(2):
                nc.tensor.matmul(pE[:], LT("cSq", ch)[:, sl], Gt[ch][:, 256:512],
                                 start=False, stop=(ch == 1))
            evict_fn(pE, hbl)

    # ------------------------------------------------------------------
    def _build_body(self, X, tc):
        nc = self.nc
        C = self._consts
        d = self.d
        C["img16_src"] = d["img16"]

        # ============ low-pass filter (f32r), 3 channels ============
        with ExitStack() as S:
            xp = S.enter_context(tc.tile_pool(name="lp_x", bufs=4))
            zp = S.enter_context(tc.tile_pool(name="lp_z", bufs=6))
            pp = S.enter_context(tc.tile_pool(name="lp_ps", bufs=1, space="PSUM"))
            ep = S.enter_context(tc.tile_pool(name="lp_ev", bufs=3))
            nev = [0]
            for c in range(3):
                def loader(xt, hbl, c=c):
                    nc.sync.dma_start(
                        xt[:], d["image"][c, hbl * 128:(hbl + 1) * 128, :].bitcast(f32r))
                def evict(pE, hbl, c=c):
                    t1 = ep.tile([128, 256], f32, name="lp_t1")
                    nc.vector.tensor_scalar(t1[:], pE[:], 0.0, 255.0,
                                            op0=ALU.max, op1=ALU.min)
                    xi = ep.tile([128, 256], mybir.dt.int32, name="lp_xi")
                    nc.vector.tensor_copy(xi[:], t1[:])
                    xr = ep.tile([128, 256], f32, name="lp_xr")
                    nc.vector.tensor_copy(xr[:], xi[:])
                    gt = ep.tile([128, 256], f32, name="lp_gt")
                    nc.vector.tensor_tensor(gt[:], xr[:], t1[:], op=ALU.is_gt)
                    l16 = ep.tile([128, WP], f16, name="lp_l16")
                    nc.vector.memset(l16[:, 0:1], 0.0)
                    nc.vector.memset(l16[:, 257:258], 0.0)
                    nc.vector.tensor_tensor(l16[:, 1:257], xr[:], gt[:],
                                            op=ALU.subtract)
                    nc.sync.dma_start(C["lpn_d"][c, hbl * 128:(hbl + 1) * 128, :],
                                      l16[:])
                self._fft_chain(tc, (xp, zp, pp),
                                src_loader=loader, mask=True, evict_fn=evict)

        if self.stop_after == 0:
            return
        # ============ conv0 ============
        sc0, sh0 = self._conv0(X, tc)

        if self.stop_after == 1:
            return
        # ============ conv1..3 ============
        sc, sh = sc0, sh0
        for k in range(1, 4):
            sc, sh = self._conv64(X, tc, k, C["y"][k - 1], C["y"][k], sc, sh)
        if self.stop_after == 2:
            return

        # ============ ch0 scale/shift broadcast to [128, 2] ============
        with ExitStack() as S:
            pp = S.enter_context(tc.tile_pool(name="bc_ps", bufs=2, space="PSUM"))
            bcp = S.enter_context(tc.tile_pool(name="bc", bufs=1))
            row = bcp.tile([1, 2], f32, name="bc_row")
            nc.sync.dma_start(row[0:1, 0:1], sc[0:1, 0:1])
            nc.sync.dma_start(row[0:1, 1:2], sh[0:1, 0:1])
            pbc = pp.tile([128, 2], f32, name="bc_mm")
            nc.tensor.matmul(pbc[:], C["ones1"][:], row[:], start=True, stop=True)
            scsh = C["cp"].tile([128, 2], f32, name="scsh0")
            nc.vector.tensor_copy(scsh[:], pbc[:])

        # ============ wm fft chain: channel 0 only (f32r) ============
        with ExitStack() as S:
            xp = S.enter_context(tc.tile_pool(name="m_x", bufs=3))
            zp = S.enter_context(tc.tile_pool(name="m_z", bufs=6))
            pp = S.enter_context(tc.tile_pool(name="m_ps", bufs=1, space="PSUM"))
            ep = S.enter_context(tc.tile_pool(name="m_ev", bufs=2))
            def loader(xt, hbl):
                raw = xp.tile([128, WP], f16, name="wm_raw")
                nc.sync.dma_start(raw[:], C["y"][3][0, hbl * 128:(hbl + 1) * 128, :])
                nc.scalar.activation(xt[:], raw[:, 1:257],
                                     FT.Relu, bias=scsh[:, 1:2], scale=scsh[:, 0:1])
            def evict(pE, hbl):
                e16 = ep.tile([128, WP], f16, name="wm_e16")
                nc.vector.memset(e16[:, 0:1], 0.0)
                nc.vector.memset(e16[:, 257:258], 0.0)
                nc.vector.tensor_copy(e16[:, 1:257], pE[:])
                nc.sync.dma_start(C["enc0_d"][0, hbl * 128:(hbl + 1) * 128, :], e16[:])
            self._fft_chain(tc, (xp, zp, pp),
                            src_loader=loader, wm=True, evict_fn=evict)

        if self.stop_after == 3:
            return
        # ============ ac conv (enc 64 + img 3 + ones 1) ============
        sc4, sh4 = self._conv64(X, tc, 4, C["y"][3], C["y"][4], sc, sh,
                                enc0=C["enc0_d"])
        if self.stop_after == 4:
            return

        # ============ final 1x1 conv ============
        with ExitStack() as S:
            bp = S.enter_context(tc.tile_pool(name="fin_b", bufs=3))
            pp = S.enter_context(tc.tile_pool(name="fin_ps", bufs=4, space="PSUM"))
            ep = S.enter_context(tc.tile_pool(name="fin_ev", bufs=3))
            sc128 = C["cp"].tile([128, 1], f32, name="fin_sc128")
            sh128 = C["cp"].tile([128, 1], f32, name="fin_sh128")
            for half in range(2):
                nc.sync.dma_start(sc128[64 * half:64 * half + 64, :], sc4[:])
                nc.sync.dma_start(sh128[64 * half:64 * half + 64, :], sh4[:])
            FR = 16   # rows per chunk (per half)
            for qi, q in enumerate(range(0, 128, FR)):
                xf = bp.tile([128, FR, WP], f16, name="fin_x")
                nc.sync.dma_start(xf[0:64, :, :], C["y"][4][:, q:q + FR, :])
                nc.sync.dma_start(xf[64:128, :, :],
                                  C["y"][4][:, 128 + q:128 + q + FR, :])
                view = xf[:, :, 1:257]
                if qi % 2 == 0:
                    nc.scalar.activation(view, view, FT.Relu,
                                         bias=sh128[:, 0:1], scale=sc128[:, 0:1])
                else:
                    nc.vector.tensor_scalar(view, view, sc128[:, 0:1],
                                            sh128[:, 0:1], op0=ALU.mult,
                                            op1=ALU.add)
                    nc.vector.tensor_scalar(view, view, 0.0, None, op0=ALU.max)
                for rr in range(0, FR, 2):
                    ptf = pp.tile([6, 512], f32, name="fin_p")
                    nc.tensor.matmul(ptf[:], C["lhsT_fin"][:],
                                     xf[:, rr:rr + 2, 1:257],
                                     start=True, stop=True)
                    osb = ep.tile([6, 512], f32, name="fin_o")
                    nc.vector.tensor_scalar(osb[:], ptf[:], C["fb6"][:, 0:1], None,
                                            op0=ALU.add)
                    nc.sync.dma_start(d["out"][:, q + rr:q + rr + 2, :],
                                      osb[0:3, :].rearrange("p (r c) -> p r c", c=256))
                    nc.sync.dma_start(d["out"][:, 128 + q + rr:128 + q + rr + 2, :],
                                      osb[3:6, :].rearrange("p (r c) -> p r c", c=256))

        # debug outputs
        for nm, src, shp, dt_ in [("lpn", C["lpn_d"], (3, H, WP), f16),
                                  ("y0", C["y"][0], (64, H, WP), f16),
                                  ("y1", C["y"][1], (64, H, WP), f16),
                                  ("y2", C["y"][2], (64, H, WP), f16),
                                  ("y3", C["y"][3], (64, H, WP), f16),
                                  ("enc0", C["enc0_d"], (1, H, WP), f16),
                                  ("y4", C["y"][4], (64, H, WP), f16)]:
            ap = self.maybe_debug(nm, shp, dt_)
            if ap is not None:
                nc.sync.dma_start(ap[:], src[:])


# ======================================================================
# harness entry point: full inputs in, full outputs out (8 cores SPMD)
# ======================================================================
from concourse.bass_utils import run_bass_kernel_spmd

_ENC = None

def _get_enc():
    global _ENC
    if _ENC is None:
        e = Enc(n_cores=8)
        e.build()
        _ENC = e
    return _ENC

def make_in_maps(inputs):
    consts = host_constants()
    g = lambda k: np.ascontiguousarray(np.asarray(inputs[k], dtype=np.float32))
    image, message = g("image"), g("message")
    w0, ws, acw = g("w0"), g("ws"), g("acw")
    fw = np.ascontiguousarray(g("fw")[:, :, 0, 0])
    shared = dict(
        b0=g("b0").reshape(64, 1), g0=g("g0").reshape(64, 1),
        be0=g("be0").reshape(64, 1), bs=g("bs").reshape(3, 64, 1),
        gs=g("gs").reshape(3, 64, 1), bes=g("bes").reshape(3, 64, 1),
        acb=g("acb").reshape(64, 1), acg=g("acg").reshape(64, 1),
        acbe=g("acbe").reshape(64, 1), fb=g("fb").reshape(3, 1), **consts)
    maps = []
    for i in range(8):
        wt = host_weights(w0, ws, acw, fw, message[i])
        im = np.ascontiguousarray(image[i])
        im16 = np.zeros((4, H, WP), np.float16)
        im16[0:3, :, 1:257] = im.astype(np.float16)
        im16[3, :, 1:257] = 1.0
        maps.append(dict(image=im, img16=im16, **wt, **shared))
    return maps

def kernel(**inputs):
    e = _get_enc()
    in_maps = make_in_maps(inputs)
    res = run_bass_kernel_spmd(e.nc, in_maps, core_ids=list(range(8)))
    out = np.stack([res.results[i]["out"] for i in range(8)], axis=0)
    return np.ascontiguousarray(out.astype(np.float32))
